# revision 1
# baseline (speedup 1.0000x reference)
"""Multi-Query Attention kernel for 8x TRN2 NeuronCores (Bass/Tile).

Problem: x[B=2, L=2048, D=2048], Wq[2048,2048], Wk/Wv[128,2048] (MQA: one
shared K/V head), 16 query heads of dim 128.

Sharding: core c in [0,8): batch b = c//4, head-group g = c%4 (4 heads,
i.e. q-channels [512g, 512g+512)). K/V replicated per core (cheap).

Device-side layout strategy (everything "transposed" so that every matmul
contraction dim lands on SBUF partitions, with zero on-device transposes of
the big tensors):
  - host passes xT = x[b].T            [D, L]  (contraction dim D on rows)
  - host passes wqT/wkT/wvT = W.T      [D, out]
  - projections compute qT/kT/vT = W @ x.T = (x@W.T).T  -> [out_ch, L]
  - scores^T tile = (kT slice).T @ qT  -> [Lk, Lq]  (contraction d=128)
  - exp on ACT engine straight out of PSUM (scale fused), no max-subtract
    (inputs are small: |scores*scale| < ~6, exp is safe in fp32)
  - out^T = (V block).T @ attn^T accumulated over Lk blocks (V natural
    [L, d] obtained via 16 cheap 128x128 PE transposes of vT)
  - softmax denominator r accumulated on the PE alongside AV: per Lk block
    one extra matmul with a full 128x128 ones stationary, which both
    reduces over the block's keys and replicates r across all partitions
    (so the final normalization is a plain DVE multiply, no broadcast)
  - phase D is software-pipelined one Lk step (AV/r matmuls for block k-1
    are emitted after the scores matmuls of block k) so the PE never
    stalls on the ACT exp latency
  - host transposes outT [512, L] back and concatenates core outputs

Matmuls run as float32r (full fp32 storage, reduced-precision multiply,
1 cycle/row at N>=256 vs 4 cycles/row for strict fp32).
"""

import os
from contextlib import ExitStack

import numpy as np

import concourse.bass as bass
import concourse.tile as tile
from concourse import bacc, masks, mybir
from concourse.bass_utils import run_bass_kernel_spmd

F32 = mybir.dt.float32
AF = mybir.ActivationFunctionType

B = 2
L = 2048
D = 2048  # d_model (contraction dim of projections)
HD = 128  # head dim
NH = 4  # heads per core
QC = NH * HD  # q-channels per core = 512
DC = D // 128  # d-model chunks of 128 = 16
NLT = 4  # l tiles of 512 (projection phase)
LKT = L // 128  # lk blocks of 128 = 16
NLQ = 4  # lq blocks of 512 (attention phase)
N_CORES = 8
SCALE = 1.0 / float(np.sqrt(HD))

# float32r: reduced-precision (tf32-like) matmul at full PE rate. Walrus
# requires every producer of an f32r-matmul operand to emit f32r, so all
# matmul-operand tiles are declared float32r and DRAM-side DMA APs are
# bitcast. Set BASS_MM_F32=1 to fall back to exact fp32 (4x slower on PE).
MM_DT = F32 if os.environ.get("BASS_MM_F32") else mybir.dt.float32r


def _mm(ap):
    return ap  # tiles already carry MM_DT


def _dr(ap):
    # bitcast a DRAM-side fp32 AP for DMA into an MM_DT tile
    return ap.bitcast(MM_DT) if MM_DT != F32 else ap


def build_kernel(ctx: ExitStack, tc: tile.TileContext, xT, wqT, wkT, wvT, bq, bk, bv, outT):
    nc = tc.nc

    persist = ctx.enter_context(tc.tile_pool(name="persist", bufs=1))
    qT = [persist.tile([128, L], MM_DT, tag=f"qT{h}", name=f"qT{h}") for h in range(NH)]  # [d, l]
    kT = persist.tile([128, L], MM_DT, tag="kT", name="kT")  # [d, l]
    vN = persist.tile([128, L], MM_DT, tag="vN", name="vN")  # block j: [:, 128j:+128] = V[128j:+128, :]
    ones = persist.tile([128, 128], F32, tag="ones", name="ones")
    ident = persist.tile([128, 128], F32, tag="ident", name="ident")
    bq_sb = persist.tile([128, NH], F32, tag="bq", name="bq")
    bk_sb = persist.tile([128, 1], F32, tag="bk", name="bk")
    bv_sb = persist.tile([128, 1], F32, tag="bv", name="bv")

    nc.vector.memset(ones[:], 1.0)
    masks.make_identity(nc, ident[:])
    nc.sync.dma_start(out=bq_sb[:], in_=bq)
    nc.sync.dma_start(out=bk_sb[:], in_=bk)
    nc.sync.dma_start(out=bv_sb[:], in_=bv)

    # ---------------- Phase B: projections qT/kT/vT = W @ x^T ----------------
    with (
        tc.tile_pool(name="wq", bufs=1) as wqp,
        tc.tile_pool(name="wkv", bufs=1) as wkvp,
        tc.tile_pool(name="xt", bufs=3) as xtp,
        tc.tile_pool(name="pj", bufs=1, space="PSUM") as pjp,
        tc.tile_pool(name="vt", bufs=1) as vtp,
    ):
        # one tile per d-chunk so each matmul waits on a single DMA sem
        wq_ch = [wqp.tile([128, QC], MM_DT, tag=f"wqc{k}", name=f"wqc{k}") for k in range(DC)]
        wk_ch = [wkvp.tile([128, HD], MM_DT, tag=f"wkc{k}", name=f"wkc{k}") for k in range(DC)]
        wv_ch = [wkvp.tile([128, HD], MM_DT, tag=f"wvc{k}", name=f"wvc{k}") for k in range(DC)]
        vT = [vtp.tile([128, 512], F32, tag=f"vT{t}", name=f"vT{t}") for t in range(NLT)]

        for lt in range(NLT):
            ls = slice(lt * 512, (lt + 1) * 512)
            # 6 concurrent PSUM accumulation groups: Q0..Q3, K, V
            psq = [pjp.tile([128, 512], F32, tag=f"pjq{t}", name=f"pjq{t}") for t in range(NH)]
            psk = pjp.tile([128, 512], F32, tag="pjk", name="pjk")
            psv = pjp.tile([128, 512], F32, tag="pjv", name="pjv")
            for k in range(DC):
                xc = xtp.tile([128, 512], MM_DT, tag=f"xt{k % 4}", name=f"xt{k % 4}")
                if lt == 0 and k == 0:
                    # split the very first transfers across 4 queues so the
                    # first matmul isn't serialized behind one queue's ramp
                    for q in range(4):
                        fs = slice(q * 128, (q + 1) * 128)
                        nc.sync.dma_start(out=xc[:, fs], in_=_dr(xT[0:128, q * 128:(q + 1) * 128]))
                        nc.sync.dma_start(out=wq_ch[0][:, fs], in_=_dr(wqT[0:128, fs]))
                else:
                    nc.sync.dma_start(out=xc[:], in_=_dr(xT[k * 128:(k + 1) * 128, ls]))
                if lt == 0:
                    # interleave weight loads with the first x tiles so the
                    # first matmul isn't gated on the whole weight transfer
                    if k > 0:
                        nc.sync.dma_start(out=wq_ch[k][:], in_=_dr(wqT[k * 128:(k + 1) * 128, :]))
                    nc.sync.dma_start(out=wk_ch[k][:], in_=_dr(wkT[k * 128:(k + 1) * 128, :]))
                    nc.sync.dma_start(out=wv_ch[k][:], in_=_dr(wvT[k * 128:(k + 1) * 128, :]))
                st = k == 0
                sp = k == DC - 1
                for t in range(NH):
                    nc.tensor.matmul(
                        psq[t][:],
                        lhsT=_mm(wq_ch[k][:, t * 128:(t + 1) * 128]),
                        rhs=_mm(xc[:]),
                        start=st,
                        stop=sp,
                    )
                nc.tensor.matmul(psk[:], lhsT=_mm(wk_ch[k][:]), rhs=_mm(xc[:]), start=st, stop=sp)
                nc.tensor.matmul(psv[:], lhsT=_mm(wv_ch[k][:]), rhs=_mm(xc[:]), start=st, stop=sp)
            for t in range(NH):
                nc.scalar.activation(qT[t][:, ls], psq[t][:], AF.Identity, bias=bq_sb[:, t:t + 1])
            nc.scalar.activation(kT[:, ls], psk[:], AF.Identity, bias=bk_sb[:, 0:1])
            nc.scalar.activation(vT[lt][:], psv[:], AF.Identity, bias=bv_sb[:, 0:1])
            # transpose this l-tile of V to natural layout right away so the
            # work overlaps the next projection tile instead of serializing
            # between the projection and attention phases
            with tc.tile_pool(name=f"tp{lt}", bufs=2, space="PSUM") as tpp:
                for jj in range(4):
                    j = lt * 4 + jj
                    pt = tpp.tile([128, 128], F32, tag="tp", name="tp")
                    nc.tensor.transpose(pt[:], vT[lt][:, jj * 128:(jj + 1) * 128], ident[:])
                    nc.scalar.activation(vN[:, j * 128:(j + 1) * 128], pt[:], AF.Identity)


    # ---------------- Phase D: attention ----------------
    # Two heads per pass: PSUM = 2x[128,1024] scores (4 banks) + 2 AV (2) +
    # 2 replicated-r (2) = 8 banks exactly. r accumulates on the PE via a
    # full 128x128 ones stationary (same stream cost as a ones *vector*,
    # and the result lands already replicated across partitions for the
    # final per-query normalization multiply).
    ones_r = persist.tile([128, 128], MM_DT, tag="ones_r", name="ones_r")
    nc.scalar.activation(ones_r[:], ones[:], AF.Identity)
    with (
        tc.tile_pool(name="sps", bufs=2, space="PSUM") as sps,  # 2 x [128,1024] = 4 banks
        tc.tile_pool(name="avp", bufs=2, space="PSUM") as avp,  # 2 x [128,512] = 2 banks
        tc.tile_pool(name="rvp", bufs=2, space="PSUM") as rvp,  # 2 x [128,512] = 2 banks
        tc.tile_pool(name="att", bufs=10) as attp,
        tc.tile_pool(name="fin", bufs=4) as finp,
    ):
        def emit_avr(p):
            """AV + r matmuls for a pending (exp'd) attention tile; when it
            closes an accumulation group, drain PSUM via ACT and finish."""
            at, psA, psR, ks, st, sp, hp, qs = (
                p["at"], p["psA"], p["psR"], p["ks"], p["st"], p["sp"], p["hp"], p["qs"],
            )
            for j in range(2):
                nc.tensor.matmul(
                    psA[j][:],
                    lhsT=_mm(vN[:, ks]),
                    rhs=_mm(at[:, j * 512:(j + 1) * 512]),
                    start=st,
                    stop=sp,
                )
            for j in range(2):
                nc.tensor.matmul(
                    psR[j][:],
                    lhsT=_mm(ones_r[:]),
                    rhs=_mm(at[:, j * 512:(j + 1) * 512]),
                    start=st,
                    stop=sp,
                )
            if sp:
                # drain the PSUM banks via ACT (idle at pass boundary) so the
                # next pass's accumulation matmuls aren't stalled on the DVE
                # finals chain
                for j in range(2):
                    h = 2 * hp + j
                    sR = finp.tile([128, 512], F32, tag="sR", name="sR")
                    nc.scalar.activation(sR[:], psR[j][:], AF.Identity)
                    sA = finp.tile([128, 512], F32, tag="sA", name="sA")
                    nc.scalar.activation(sA[:], psA[j][:], AF.Identity)
                    rinv = finp.tile([128, 512], F32, tag="rinv", name="rinv")
                    nc.vector.reciprocal(rinv[:], sR[:])
                    ot = finp.tile([128, 512], F32, tag="ot", name="ot")
                    nc.vector.tensor_mul(ot[:], sA[:], rinv[:])
                    nc.sync.dma_start(out=outT[h * 128:(h + 1) * 128, qs], in_=ot[:])

        pend = []
        for lq in range(NLQ):
            qs = slice(lq * 512, (lq + 1) * 512)
            for hp in range(2):  # head pairs
                psA = [avp.tile([128, 512], F32, tag="av", name="av") for _ in range(2)]
                psR = [rvp.tile([128, 512], F32, tag="rv", name="rv") for _ in range(2)]
                for lk in range(LKT):
                    ks = slice(lk * 128, (lk + 1) * 128)
                    ss = sps.tile([128, 1024], F32, tag="sps", name="sps")
                    for j in range(2):
                        nc.tensor.matmul(
                            ss[:, j * 512:(j + 1) * 512],
                            lhsT=_mm(kT[:, ks]),
                            rhs=_mm(qT[2 * hp + j][:, qs]),
                            start=True,
                            stop=True,
                        )
                    at = attp.tile([128, 1024], MM_DT, tag="att", name="att")
                    nc.scalar.activation(at[:], ss[:], AF.Exp, scale=SCALE)
                    # software pipeline (depth 2): consume an OLDER chunk's
                    # exp so the PE never waits on ACT latency, and each
                    # pass's finals land well before the next pass's first
                    # accumulation matmuls need the PSUM banks back
                    pend.append(dict(
                        at=at, psA=psA, psR=psR, ks=ks,
                        st=(lk == 0), sp=(lk == LKT - 1), hp=hp, qs=qs,
                    ))
                    if len(pend) > 4:
                        emit_avr(pend.pop(0))
        while pend:
            emit_avr(pend.pop(0))


_NC_CACHE = None


def build_nc():
    global _NC_CACHE
    if _NC_CACHE is not None:
        return _NC_CACHE
    nc = bacc.Bacc("TRN2", target_bir_lowering=False, debug=False)
    xT = nc.dram_tensor("xT", [D, L], F32, kind="ExternalInput").ap()
    wqT = nc.dram_tensor("wqT", [D, QC], F32, kind="ExternalInput").ap()
    wkT = nc.dram_tensor("wkT", [D, HD], F32, kind="ExternalInput").ap()
    wvT = nc.dram_tensor("wvT", [D, HD], F32, kind="ExternalInput").ap()
    bq = nc.dram_tensor("bq", [128, NH], F32, kind="ExternalInput").ap()
    bk = nc.dram_tensor("bk", [128, 1], F32, kind="ExternalInput").ap()
    bv = nc.dram_tensor("bv", [128, 1], F32, kind="ExternalInput").ap()
    outT = nc.dram_tensor("outT", [QC, L], F32, kind="ExternalOutput").ap()
    with tile.TileContext(nc) as tc, ExitStack() as ctx:
        build_kernel(ctx, tc, xT, wqT, wkT, wvT, bq, bk, bv, outT)
    nc.compile()
    _NC_CACHE = nc
    return nc


def make_in_maps(x, Wq_w, Wq_b, Wk_w, Wk_b, Wv_w, Wv_b):
    """Host-side sharding/relayout. Returns one input map per core."""
    x = np.asarray(x, dtype=np.float32)
    Wq_w = np.asarray(Wq_w, dtype=np.float32)
    Wq_b = np.asarray(Wq_b, dtype=np.float32)
    Wk_w = np.asarray(Wk_w, dtype=np.float32)
    Wk_b = np.asarray(Wk_b, dtype=np.float32)
    Wv_w = np.asarray(Wv_w, dtype=np.float32)
    Wv_b = np.asarray(Wv_b, dtype=np.float32)

    xTs = [np.ascontiguousarray(x[b].T) for b in range(B)]
    wkT = np.ascontiguousarray(Wk_w.T)
    wvT = np.ascontiguousarray(Wv_w.T)
    bk = np.ascontiguousarray(Wk_b.reshape(128, 1))
    bv = np.ascontiguousarray(Wv_b.reshape(128, 1))
    in_maps = []
    for c in range(N_CORES):
        b, g = divmod(c, B * 2)  # b = c // 4, g = c % 4
        wqT_g = np.ascontiguousarray(Wq_w[g * QC:(g + 1) * QC, :].T)
        bq_g = np.ascontiguousarray(Wq_b[g * QC:(g + 1) * QC].reshape(NH, 128).T)
        in_maps.append(
            {
                "xT": xTs[b],
                "wqT": wqT_g,
                "wkT": wkT,
                "wvT": wvT,
                "bq": bq_g,
                "bk": bk,
                "bv": bv,
            }
        )
    return in_maps


def assemble_output(results):
    out = np.empty((B, L, D), dtype=np.float32)
    for c in range(N_CORES):
        b, g = divmod(c, B * 2)
        out[b, :, g * QC:(g + 1) * QC] = results[c]["outT"].T
    return out


def kernel(**inputs) -> np.ndarray:
    nc = build_nc()
    in_maps = make_in_maps(**inputs)
    res = run_bass_kernel_spmd(nc, in_maps, core_ids=list(range(N_CORES)))
    return assemble_output(res.results)



# revision 2
# speedup vs baseline: 1.0991x; 1.0991x over previous
"""Multi-Query Attention kernel for 8x TRN2 NeuronCores (Bass/Tile).

Problem: x[B=2, L=2048, D=2048], Wq[2048,2048], Wk/Wv[128,2048] (MQA: one
shared K/V head), 16 query heads of dim 128.

Sharding: core c in [0,8): batch b = c//4, head-group g = c%4 (4 heads,
i.e. q-channels [512g, 512g+512)). K/V replicated per core (cheap).

Device-side layout strategy (everything "transposed" so that every matmul
contraction dim lands on SBUF partitions, with zero on-device transposes of
the big tensors):
  - host passes x tiles pre-transposed + pre-tiled so every DMA is a single
    contiguous packet: xt4[k, lt] = x[b].T[128k:+128, 512lt:+512]  (bf16)
  - host passes wq4/wk4/wv4 = W.T pre-tiled per 128-row d-chunk     (bf16)
  - projections compute qT/kT/vT = W @ x.T = (x@W.T).T -> [out_ch, L]
  - scores^T tile = (kT slice).T @ qT -> [Lk, Lq]  (contraction d=128)
  - exp on ACT engine straight out of PSUM (scale fused), no max-subtract
    (inputs are small: |scores*scale| < ~6, exp is safe)
  - out^T = (V block).T @ attn^T accumulated over Lk blocks (V natural
    [L, d] obtained via 16 cheap 128x128 PE transposes of vT)
  - softmax denominator r accumulated on the PE alongside AV: per Lk block
    one extra matmul with a full 128x128 ones stationary, which both
    reduces over the block's keys and replicates r across all partitions
    (so the final normalization is a plain DVE multiply, no broadcast)
  - phase D is software-pipelined one Lk step (AV/r matmuls for block k-1
    are emitted after the scores matmuls of block k) so the PE never
    stalls on the ACT exp latency
  - output written as contiguous [128,512] fp32 blocks (single-packet
    DMAs); host reassembles + concatenates core outputs

All matmul operands are bfloat16 (PSUM accumulation stays fp32): same
1 cycle/row PE stream rate as float32r, but LDWEIGHTS is ~4x cheaper
(fully hidden under the previous matmul) and input DMA bytes halve.
"""

from contextlib import ExitStack

import ml_dtypes
import numpy as np

import concourse.bass as bass
import concourse.tile as tile
from concourse import bacc, masks, mybir
from concourse.bass_utils import run_bass_kernel_spmd

F32 = mybir.dt.float32
BF16 = mybir.dt.bfloat16
AF = mybir.ActivationFunctionType

B = 2
L = 2048
D = 2048  # d_model (contraction dim of projections)
HD = 128  # head dim
NH = 4  # heads per core
QC = NH * HD  # q-channels per core = 512
DC = D // 128  # d-model chunks of 128 = 16
NLT = 4  # l tiles of 512 (projection phase)
LKT = L // 128  # lk blocks of 128 = 16
NLQ = 4  # lq blocks of 512 (attention phase)
N_CORES = 8
SCALE = 1.0 / float(np.sqrt(HD))


def build_kernel(ctx: ExitStack, tc: tile.TileContext, xt4, wq4, wk4, wv4, bq, bk, bv, out4):
    nc = tc.nc

    persist = ctx.enter_context(tc.tile_pool(name="persist", bufs=1))
    qT = [persist.tile([128, L], BF16, tag=f"qT{h}", name=f"qT{h}") for h in range(NH)]  # [d, l]
    kT = persist.tile([128, L], BF16, tag="kT", name="kT")  # [d, l]
    vN = persist.tile([128, L], BF16, tag="vN", name="vN")  # block j: [:, 128j:+128] = V[128j:+128, :]
    ident = persist.tile([128, 128], BF16, tag="ident", name="ident")
    ones_r = persist.tile([128, 128], BF16, tag="ones_r", name="ones_r")
    bq_sb = persist.tile([128, NH], F32, tag="bq", name="bq")
    bk_sb = persist.tile([128, 1], F32, tag="bk", name="bk")
    bv_sb = persist.tile([128, 1], F32, tag="bv", name="bv")

    nc.vector.memset(ones_r[:], 1.0)
    masks.make_identity(nc, ident[:])
    nc.sync.dma_start(out=bq_sb[:], in_=bq)
    nc.sync.dma_start(out=bk_sb[:], in_=bk)
    nc.sync.dma_start(out=bv_sb[:], in_=bv)

    # ---------------- Phase B: projections qT/kT/vT = W @ x^T ----------------
    with (
        tc.tile_pool(name="wq", bufs=1) as wqp,
        tc.tile_pool(name="wkv", bufs=1) as wkvp,
        tc.tile_pool(name="xt", bufs=8) as xtp,
        tc.tile_pool(name="pj", bufs=1, space="PSUM") as pjp,
        tc.tile_pool(name="vt", bufs=1) as vtp,
    ):
        # one tile per d-chunk so each matmul waits on a single DMA sem
        wq_ch = [wqp.tile([128, QC], BF16, tag=f"wqc{k}", name=f"wqc{k}") for k in range(DC)]
        wk_ch = [wkvp.tile([128, HD], BF16, tag=f"wkc{k}", name=f"wkc{k}") for k in range(DC)]
        wv_ch = [wkvp.tile([128, HD], BF16, tag=f"wvc{k}", name=f"wvc{k}") for k in range(DC)]
        vT = [vtp.tile([128, 512], BF16, tag=f"vT{t}", name=f"vT{t}") for t in range(NLT)]

        for lt in range(NLT):
            ls = slice(lt * 512, (lt + 1) * 512)
            # 6 concurrent PSUM accumulation groups: Q0..Q3, K, V
            psq = [pjp.tile([128, 512], F32, tag=f"pjq{t}", name=f"pjq{t}") for t in range(NH)]
            psk = pjp.tile([128, 512], F32, tag="pjk", name="pjk")
            psv = pjp.tile([128, 512], F32, tag="pjv", name="pjv")
            for k in range(DC):
                xc = xtp.tile([128, 512], BF16, tag=f"xt{k % 8}", name=f"xt{k % 8}")
                nc.sync.dma_start(out=xc[:], in_=xt4[k, lt])
                if lt == 0:
                    # interleave weight loads with the x tiles so the first
                    # matmul isn't gated on the whole weight transfer
                    nc.sync.dma_start(out=wq_ch[k][:], in_=wq4[k])
                    nc.sync.dma_start(out=wk_ch[k][:], in_=wk4[k])
                    nc.sync.dma_start(out=wv_ch[k][:], in_=wv4[k])
                st = k == 0
                sp = k == DC - 1
                for t in range(NH):
                    nc.tensor.matmul(
                        psq[t][:],
                        lhsT=wq_ch[k][:, t * 128:(t + 1) * 128],
                        rhs=xc[:],
                        start=st,
                        stop=sp,
                    )
                nc.tensor.matmul(psk[:], lhsT=wk_ch[k][:], rhs=xc[:], start=st, stop=sp)
                nc.tensor.matmul(psv[:], lhsT=wv_ch[k][:], rhs=xc[:], start=st, stop=sp)
            for t in range(NH):
                nc.scalar.activation(qT[t][:, ls], psq[t][:], AF.Identity, bias=bq_sb[:, t:t + 1])
            nc.scalar.activation(kT[:, ls], psk[:], AF.Identity, bias=bk_sb[:, 0:1])
            nc.scalar.activation(vT[lt][:], psv[:], AF.Identity, bias=bv_sb[:, 0:1])
            # transpose this l-tile of V to natural layout right away so the
            # work overlaps the next projection tile instead of serializing
            # between the projection and attention phases
            with tc.tile_pool(name=f"tp{lt}", bufs=2, space="PSUM") as tpp:
                for jj in range(4):
                    j = lt * 4 + jj
                    pt = tpp.tile([128, 128], BF16, tag="tp", name="tp")
                    nc.tensor.transpose(pt[:], vT[lt][:, jj * 128:(jj + 1) * 128], ident[:])
                    nc.scalar.activation(vN[:, j * 128:(j + 1) * 128], pt[:], AF.Identity)

    # ---------------- Phase D: attention ----------------
    # Two heads per pass: PSUM = 2x[128,1024] scores (4 banks) + 2 AV (2) +
    # 2 replicated-r (2) = 8 banks exactly. r accumulates on the PE via a
    # full 128x128 ones stationary (same stream cost as a ones *vector*,
    # and the result lands already replicated across partitions for the
    # final per-query normalization multiply).
    with (
        tc.tile_pool(name="sps", bufs=2, space="PSUM") as sps,  # 2 x [128,1024] = 4 banks
        tc.tile_pool(name="avp", bufs=2, space="PSUM") as avp,  # 2 x [128,512] = 2 banks
        tc.tile_pool(name="rvp", bufs=2, space="PSUM") as rvp,  # 2 x [128,512] = 2 banks
        tc.tile_pool(name="att", bufs=10) as attp,
        tc.tile_pool(name="fin", bufs=4) as finp,
    ):
        def emit_avr(p):
            """AV + r matmuls for a pending (exp'd) attention tile; when it
            closes an accumulation group, drain PSUM via ACT and finish."""
            at, psA, psR, ks, st, sp, hp, lq = (
                p["at"], p["psA"], p["psR"], p["ks"], p["st"], p["sp"], p["hp"], p["lq"],
            )
            for j in range(2):
                nc.tensor.matmul(
                    psA[j][:],
                    lhsT=vN[:, ks],
                    rhs=at[:, j * 512:(j + 1) * 512],
                    start=st,
                    stop=sp,
                )
            for j in range(2):
                nc.tensor.matmul(
                    psR[j][:],
                    lhsT=ones_r[:],
                    rhs=at[:, j * 512:(j + 1) * 512],
                    start=st,
                    stop=sp,
                )
            if sp:
                # drain the PSUM banks via ACT (idle at pass boundary) so the
                # next pass's accumulation matmuls aren't stalled on the DVE
                # finals chain
                for j in range(2):
                    h = 2 * hp + j
                    sR = finp.tile([128, 512], F32, tag="sR", name="sR")
                    nc.scalar.activation(sR[:], psR[j][:], AF.Identity)
                    sA = finp.tile([128, 512], F32, tag="sA", name="sA")
                    nc.scalar.activation(sA[:], psA[j][:], AF.Identity)
                    rinv = finp.tile([128, 512], F32, tag="rinv", name="rinv")
                    nc.vector.reciprocal_approx_fast(out=rinv[:], in_=sR[:])
                    ot = finp.tile([128, 512], F32, tag="ot", name="ot")
                    nc.vector.tensor_mul(ot[:], sA[:], rinv[:])
                    nc.sync.dma_start(out=out4[lq, h], in_=ot[:])

        pend = []
        for lq in range(NLQ):
            qs = slice(lq * 512, (lq + 1) * 512)
            for hp in range(2):  # head pairs
                psA = [avp.tile([128, 512], F32, tag="av", name="av") for _ in range(2)]
                psR = [rvp.tile([128, 512], F32, tag="rv", name="rv") for _ in range(2)]
                for lk in range(LKT):
                    ks = slice(lk * 128, (lk + 1) * 128)
                    ss = sps.tile([128, 1024], F32, tag="sps", name="sps")
                    for j in range(2):
                        nc.tensor.matmul(
                            ss[:, j * 512:(j + 1) * 512],
                            lhsT=kT[:, ks],
                            rhs=qT[2 * hp + j][:, qs],
                            start=True,
                            stop=True,
                        )
                    at = attp.tile([128, 1024], BF16, tag="att", name="att")
                    nc.scalar.activation(at[:], ss[:], AF.Exp, scale=SCALE)
                    # software pipeline (depth 2): consume an OLDER chunk's
                    # exp so the PE never waits on ACT latency, and each
                    # pass's finals land well before the next pass's first
                    # accumulation matmuls need the PSUM banks back
                    pend.append(dict(
                        at=at, psA=psA, psR=psR, ks=ks,
                        st=(lk == 0), sp=(lk == LKT - 1), hp=hp, lq=lq,
                    ))
                    if len(pend) > 4:
                        emit_avr(pend.pop(0))
        while pend:
            emit_avr(pend.pop(0))


_NC_CACHE = None


def build_nc():
    global _NC_CACHE
    if _NC_CACHE is not None:
        return _NC_CACHE
    nc = bacc.Bacc("TRN2", target_bir_lowering=False, debug=False)
    xt4 = nc.dram_tensor("xt4", [DC, NLT, 128, 512], BF16, kind="ExternalInput").ap()
    wq4 = nc.dram_tensor("wq4", [DC, 128, QC], BF16, kind="ExternalInput").ap()
    wk4 = nc.dram_tensor("wk4", [DC, 128, HD], BF16, kind="ExternalInput").ap()
    wv4 = nc.dram_tensor("wv4", [DC, 128, HD], BF16, kind="ExternalInput").ap()
    bq = nc.dram_tensor("bq", [128, NH], F32, kind="ExternalInput").ap()
    bk = nc.dram_tensor("bk", [128, 1], F32, kind="ExternalInput").ap()
    bv = nc.dram_tensor("bv", [128, 1], F32, kind="ExternalInput").ap()
    out4 = nc.dram_tensor("out4", [NLQ, NH, 128, 512], F32, kind="ExternalOutput").ap()
    with tile.TileContext(nc) as tc, ExitStack() as ctx:
        build_kernel(ctx, tc, xt4, wq4, wk4, wv4, bq, bk, bv, out4)
    nc.compile()
    _NC_CACHE = nc
    return nc


def _bf16(a):
    return np.ascontiguousarray(a.astype(ml_dtypes.bfloat16))


def make_in_maps(x, Wq_w, Wq_b, Wk_w, Wk_b, Wv_w, Wv_b):
    """Host-side sharding/relayout. Returns one input map per core."""
    x = np.asarray(x, dtype=np.float32)
    Wq_w = np.asarray(Wq_w, dtype=np.float32)
    Wq_b = np.asarray(Wq_b, dtype=np.float32)
    Wk_w = np.asarray(Wk_w, dtype=np.float32)
    Wk_b = np.asarray(Wk_b, dtype=np.float32)
    Wv_w = np.asarray(Wv_w, dtype=np.float32)
    Wv_b = np.asarray(Wv_b, dtype=np.float32)

    # xt4[k, lt, d, l] = x[b].T[128k+d, 512lt+l]: every [128, 512] tile is
    # one contiguous 128KB DMA packet
    xt4s = [
        _bf16(x[b].T.reshape(DC, 128, NLT, 512).transpose(0, 2, 1, 3))
        for b in range(B)
    ]
    wk4 = _bf16(Wk_w.T.reshape(DC, 128, HD))
    wv4 = _bf16(Wv_w.T.reshape(DC, 128, HD))
    bk = np.ascontiguousarray(Wk_b.reshape(128, 1))
    bv = np.ascontiguousarray(Wv_b.reshape(128, 1))
    in_maps = []
    for c in range(N_CORES):
        b, g = divmod(c, B * 2)  # b = c // 4, g = c % 4
        wq4_g = _bf16(Wq_w[g * QC:(g + 1) * QC, :].T.reshape(DC, 128, QC))
        bq_g = np.ascontiguousarray(Wq_b[g * QC:(g + 1) * QC].reshape(NH, 128).T)
        in_maps.append(
            {
                "xt4": xt4s[b],
                "wq4": wq4_g,
                "wk4": wk4,
                "wv4": wv4,
                "bq": bq_g,
                "bk": bk,
                "bv": bv,
            }
        )
    return in_maps


def assemble_output(results):
    out = np.empty((B, L, D), dtype=np.float32)
    for c in range(N_CORES):
        b, g = divmod(c, B * 2)
        o4 = np.asarray(results[c]["out4"])  # [lq, h, d, q]
        out[b, :, g * QC:(g + 1) * QC] = o4.transpose(0, 3, 1, 2).reshape(L, QC)
    return out


def kernel(**inputs) -> np.ndarray:
    nc = build_nc()
    in_maps = make_in_maps(**inputs)
    res = run_bass_kernel_spmd(nc, in_maps, core_ids=list(range(N_CORES)))
    return assemble_output(res.results)


# revision 6
# speedup vs baseline: 1.1896x; 1.0824x over previous
"""Multi-Query Attention kernel for 8x TRN2 NeuronCores (Bass/Tile).

Problem: x[B=2, L=2048, D=2048], Wq[2048,2048], Wk/Wv[128,2048] (MQA: one
shared K/V head), 16 query heads of dim 128.

Sharding: core c in [0,8): batch b = c//4, head-group g = c%4 (4 heads,
i.e. q-channels [512g, 512g+512)). K/V replicated per core (cheap).

Device-side layout strategy (everything "transposed" so that every matmul
contraction dim lands on SBUF partitions, with zero on-device transposes of
the big tensors):
  - host passes xtb = x[b].T bf16 [D, L]: each 128-row d-chunk is one
    contiguous 512KB DMA (4KB per partition line); chunk 0 is split into
    four [128,512] tiles so the very first matmul starts sooner
  - host passes weights transposed + pair-packed so every weight DMA is one
    contiguous >=2KB-line packet: wq8[kk] holds d-chunks 2kk,2kk+1 side by
    side; wk8/wv8[kk] hold 8 chunks of 128 cols each
  - projections compute qT/kT/vT = W @ x.T = (x@W.T).T -> [out_ch, L]
  - scores^T tile = (kT slice).T @ qT -> [Lk, Lq]  (contraction d=128)
  - exp on ACT engine straight out of PSUM (scale fused), no max-subtract
    (inputs are small: |scores*scale| < ~6, exp is safe)
  - out^T = (V block).T @ attn^T accumulated over Lk blocks (V natural
    [L, d] obtained via 16 cheap 128x128 PE transposes of vT)
  - softmax denominator r: exp tiles are pre-accumulated 4 lk-blocks at a
    time on the (otherwise idle) DVE with a bf16 add tree, then ONE ones-
    stationary matmul per group reduces over partitions and replicates r
    -> 4x fewer PE rows for r than a per-block ones-matmul
  - AV matmuls and r-groups are software-pipelined behind the scores
    matmuls so the PE never stalls on ACT/DVE latency
  - output written as contiguous [128,512] fp32 blocks (single-packet
    DMAs); host reassembles + concatenates core outputs

All matmul operands are bfloat16 (PSUM accumulation stays fp32): same
1 cycle/row PE stream rate as float32r, but LDWEIGHTS is ~4x cheaper
(fully hidden under the previous matmul) and input DMA bytes halve.
"""

from contextlib import ExitStack

import ml_dtypes
import numpy as np

import concourse.bass as bass
import concourse.tile as tile
from concourse import bacc, masks, mybir
from concourse.bass_utils import run_bass_kernel_spmd

F32 = mybir.dt.float32
BF16 = mybir.dt.bfloat16
AF = mybir.ActivationFunctionType

B = 2
L = 2048
D = 2048  # d_model (contraction dim of projections)
HD = 128  # head dim
NH = 4  # heads per core
QC = NH * HD  # q-channels per core = 512
DC = D // 128  # d-model chunks of 128 = 16
NLT = 4  # l tiles of 512 (projection phase)
LKT = L // 128  # lk blocks of 128 = 16
NLQ = 4  # lq blocks of 512 (attention phase)
N_CORES = 8
SCALE = 1.0 / float(np.sqrt(HD))


def build_kernel(ctx: ExitStack, tc: tile.TileContext, xtb, wq8, wk8, wv8, bq, bk, bv, out4):
    nc = tc.nc

    persist = ctx.enter_context(tc.tile_pool(name="persist", bufs=1))
    qT = [persist.tile([128, L], BF16, tag=f"qT{h}", name=f"qT{h}") for h in range(NH)]  # [d, l]
    kT = persist.tile([128, L], BF16, tag="kT", name="kT")  # [d, l]
    vN = persist.tile([128, L], BF16, tag="vN", name="vN")  # block j: [:, 128j:+128] = V[128j:+128, :]
    ident = persist.tile([128, 128], BF16, tag="ident", name="ident")
    ones_r = persist.tile([128, 128], BF16, tag="ones_r", name="ones_r")
    bq_sb = persist.tile([128, NH], F32, tag="bq", name="bq")
    bk_sb = persist.tile([128, 1], F32, tag="bk", name="bk")
    bv_sb = persist.tile([128, 1], F32, tag="bv", name="bv")

    nc.vector.memset(ones_r[:], 1.0)
    masks.make_identity(nc, ident[:])
    nc.sync.dma_start(out=bq_sb[:], in_=bq)
    nc.sync.dma_start(out=bk_sb[:], in_=bk)
    nc.sync.dma_start(out=bv_sb[:], in_=bv)

    # ---------------- Phase B: projections qT/kT/vT = W @ x^T ----------------
    with (
        tc.tile_pool(name="wq", bufs=1) as wqp,
        tc.tile_pool(name="wkv", bufs=1) as wkvp,
        tc.tile_pool(name="xt", bufs=1) as xtp,
        tc.tile_pool(name="pj", bufs=1, space="PSUM") as pjp,
        tc.tile_pool(name="vt", bufs=1) as vtp,
    ):
        wq_p = [wqp.tile([128, 1024], BF16, tag=f"wqp{kk}", name=f"wqp{kk}") for kk in range(8)]
        wk_p = [wkvp.tile([128, 1024], BF16, tag=f"wkp{kk}", name=f"wkp{kk}") for kk in range(2)]
        wv_p = [wkvp.tile([128, 1024], BF16, tag=f"wvp{kk}", name=f"wvp{kk}") for kk in range(2)]
        x0 = [xtp.tile([128, 512], BF16, tag=f"x0_{t}", name=f"x0_{t}") for t in range(NLT)]
        xb = [xtp.tile([128, 2048], BF16, tag=f"xb{k}", name=f"xb{k}") for k in range(1, DC)]
        vT = [vtp.tile([128, 512], BF16, tag=f"vT{t}", name=f"vT{t}") for t in range(NLT)]

        def wq_sl(k, t):  # stationary [128, 128] for d-chunk k, head t
            base = (k % 2) * 512 + t * 128
            return wq_p[k // 2][:, base:base + 128]

        def wk_sl(k):
            return wk_p[k // 8][:, (k % 8) * 128:(k % 8) * 128 + 128]

        def wv_sl(k):
            return wv_p[k // 8][:, (k % 8) * 128:(k % 8) * 128 + 128]

        def x_sl(k, lt):
            if k == 0:
                return x0[lt][:]
            return xb[k - 1][:, lt * 512:(lt + 1) * 512]

        # issue all input DMAs up front, ordered so arrival tracks first-use:
        # lt=0's k-loop consumes chunk k at ~1.3us intervals
        nc.sync.dma_start(out=x0[0][:], in_=xtb[0:128, 0:512])
        nc.sync.dma_start(out=wq_p[0][:], in_=wq8[0])
        nc.sync.dma_start(out=wk_p[0][:], in_=wk8[0])
        nc.sync.dma_start(out=wv_p[0][:], in_=wv8[0])
        for t in range(1, NLT):
            nc.sync.dma_start(out=x0[t][:], in_=xtb[0:128, t * 512:(t + 1) * 512])
        for k in range(1, DC):
            nc.sync.dma_start(out=xb[k - 1][:], in_=xtb[k * 128:(k + 1) * 128, :])
            if k % 2 == 0 and k // 2 < 8:
                nc.sync.dma_start(out=wq_p[k // 2][:], in_=wq8[k // 2])
            if k == 7:
                nc.sync.dma_start(out=wk_p[1][:], in_=wk8[1])
                nc.sync.dma_start(out=wv_p[1][:], in_=wv8[1])

        for lt in range(NLT):
            ls = slice(lt * 512, (lt + 1) * 512)
            # 6 concurrent PSUM accumulation groups: Q0..Q3, K, V
            psq = [pjp.tile([128, 512], F32, tag=f"pjq{t}", name=f"pjq{t}") for t in range(NH)]
            psk = pjp.tile([128, 512], F32, tag="pjk", name="pjk")
            psv = pjp.tile([128, 512], F32, tag="pjv", name="pjv")
            for k in range(DC):
                st = k == 0
                sp = k == DC - 1
                xs = x_sl(k, lt)
                for t in range(NH):
                    nc.tensor.matmul(psq[t][:], lhsT=wq_sl(k, t), rhs=xs, start=st, stop=sp)
                nc.tensor.matmul(psk[:], lhsT=wk_sl(k), rhs=xs, start=st, stop=sp)
                nc.tensor.matmul(psv[:], lhsT=wv_sl(k), rhs=xs, start=st, stop=sp)
            for t in range(NH):
                nc.scalar.activation(qT[t][:, ls], psq[t][:], AF.Identity, bias=bq_sb[:, t:t + 1])
            nc.scalar.activation(kT[:, ls], psk[:], AF.Identity, bias=bk_sb[:, 0:1])
            nc.scalar.activation(vT[lt][:], psv[:], AF.Identity, bias=bv_sb[:, 0:1])
            # transpose this l-tile of V to natural layout right away so the
            # work overlaps the next projection tile instead of serializing
            # between the projection and attention phases
            with tc.tile_pool(name=f"tp{lt}", bufs=2, space="PSUM") as tpp:
                for jj in range(4):
                    j = lt * 4 + jj
                    pt = tpp.tile([128, 128], BF16, tag="tp", name="tp")
                    nc.tensor.transpose(pt[:], vT[lt][:, jj * 128:(jj + 1) * 128], ident[:])
                    nc.scalar.activation(vN[:, j * 128:(j + 1) * 128], pt[:], AF.Identity)

    # ---------------- Phase D: attention ----------------
    # Two heads per pass: PSUM = 2x[128,1024] scores (4 banks) + 2 AV (2) +
    # 2 r (2) = 8 banks exactly. The softmax denominator r is computed by
    # first tree-summing each group of 4 exp tiles on the DVE (bf16), then
    # one ones-stationary matmul per group both reduces over the group's
    # 128 partitions and replicates r across partitions (so the final
    # normalization is a plain DVE multiply, no broadcast).
    with (
        tc.tile_pool(name="sps", bufs=2, space="PSUM") as sps,  # 2 x [128,1024] = 4 banks
        tc.tile_pool(name="avp", bufs=2, space="PSUM") as avp,  # 2 x [128,512] = 2 banks
        tc.tile_pool(name="rvp", bufs=2, space="PSUM") as rvp,  # 2 x [128,512] = 2 banks
        tc.tile_pool(name="att", bufs=12) as attp,
        tc.tile_pool(name="rac", bufs=2) as racp,
        tc.tile_pool(name="fin", bufs=4) as finp,
    ):
        def emit_av(p):
            """AV matmuls for a pending (exp'd) attention tile."""
            at, psA, ks, st, sp = p["at"], p["psA"], p["ks"], p["st"], p["sp"]
            for j in range(2):
                nc.tensor.matmul(
                    psA[j][:],
                    lhsT=vN[:, ks],
                    rhs=at[:, j * 512:(j + 1) * 512],
                    start=st,
                    stop=sp,
                )
            if sp:
                p["done"][0] = True

        def emit_rgroup(p):
            """bf16 DVE add-tree over a group of 4 exp tiles, then one
            ones-stationary matmul per head accumulating into psR."""
            ats, psR, g = p["ats"], p["psR"], p["g"]
            t0 = racp.tile([128, 1024], BF16, tag="t0", name="t0")
            t1 = racp.tile([128, 1024], BF16, tag="t1", name="t1")
            acc = racp.tile([128, 1024], BF16, tag="acc", name="acc")
            nc.vector.tensor_add(t0[:], ats[0][:], ats[1][:])
            nc.vector.tensor_add(t1[:], ats[2][:], ats[3][:])
            nc.vector.tensor_add(acc[:], t0[:], t1[:])
            for j in range(2):
                nc.tensor.matmul(
                    psR[j][:],
                    lhsT=ones_r[:],
                    rhs=acc[:, j * 512:(j + 1) * 512],
                    start=g == 0,
                    stop=g == 3,
                )
            if g == 3:
                p["done"][0] = True

        def emit_finals(p):
            """Drain PSUM via ACT, normalize on DVE, store output."""
            psA, psR, hp, lq = p["psA"], p["psR"], p["hp"], p["lq"]
            for j in range(2):
                h = 2 * hp + j
                sR = finp.tile([128, 512], F32, tag="sR", name="sR")
                nc.scalar.activation(sR[:], psR[j][:], AF.Identity)
                sA = finp.tile([128, 512], F32, tag="sA", name="sA")
                nc.scalar.activation(sA[:], psA[j][:], AF.Identity)
                rinv = finp.tile([128, 512], F32, tag="rinv", name="rinv")
                nc.vector.reciprocal_approx_fast(out=rinv[:], in_=sR[:])
                ot = finp.tile([128, 512], F32, tag="ot", name="ot")
                nc.vector.tensor_mul(ot[:], sA[:], rinv[:])
                nc.sync.dma_start(out=out4[lq, h], in_=ot[:])

        pendAV = []
        pendR = []
        pendF = []

        def pump(av_keep, r_keep):
            while len(pendAV) > av_keep:
                emit_av(pendAV.pop(0))
            while len(pendR) > r_keep:
                emit_rgroup(pendR.pop(0))
            # finals for a pass go out once its last AV (sp) and last
            # r-group (g=3) have both been emitted
            while pendF:
                f = pendF[0]
                if f["navd"][0] and f["nrgd"][0]:
                    emit_finals(pendF.pop(0))
                else:
                    break

        for lq in range(NLQ):
            qs = slice(lq * 512, (lq + 1) * 512)
            for hp in range(2):  # head pairs
                psA = [avp.tile([128, 512], F32, tag="av", name="av") for _ in range(2)]
                psR = [rvp.tile([128, 512], F32, tag="rv", name="rv") for _ in range(2)]
                avd, rgd = [False], [False]
                pendF.append(dict(psA=psA, psR=psR, hp=hp, lq=lq, navd=avd, nrgd=rgd))
                for g in range(4):
                    g_ats = []
                    for li in range(4):
                        lk = 4 * g + li
                        ks = slice(lk * 128, (lk + 1) * 128)
                        ss = sps.tile([128, 1024], F32, tag="sps", name="sps")
                        for j in range(2):
                            nc.tensor.matmul(
                                ss[:, j * 512:(j + 1) * 512],
                                lhsT=kT[:, ks],
                                rhs=qT[2 * hp + j][:, qs],
                                start=True,
                                stop=True,
                            )
                        at = attp.tile([128, 1024], BF16, tag="att", name="att")
                        nc.scalar.activation(at[:], ss[:], AF.Exp, scale=SCALE)
                        g_ats.append(at)
                        pendAV.append(dict(
                            at=at, psA=psA, ks=ks,
                            st=(lk == 0), sp=(lk == LKT - 1), done=avd,
                        ))
                        pump(4, 2)
                    pendR.append(dict(ats=g_ats, psR=psR, g=g, done=rgd))
        # flush
        while pendAV or pendR or pendF:
            if pendAV:
                emit_av(pendAV.pop(0))
            if pendR:
                emit_rgroup(pendR.pop(0))
            while pendF:
                f = pendF[0]
                if f["navd"][0] and f["nrgd"][0]:
                    emit_finals(pendF.pop(0))
                else:
                    break


_NC_CACHE = None


def build_nc():
    global _NC_CACHE
    if _NC_CACHE is not None:
        return _NC_CACHE
    nc = bacc.Bacc("TRN2", target_bir_lowering=False, debug=False)
    xtb = nc.dram_tensor("xtb", [D, L], BF16, kind="ExternalInput").ap()
    wq8 = nc.dram_tensor("wq8", [8, 128, 1024], BF16, kind="ExternalInput").ap()
    wk8 = nc.dram_tensor("wk8", [2, 128, 1024], BF16, kind="ExternalInput").ap()
    wv8 = nc.dram_tensor("wv8", [2, 128, 1024], BF16, kind="ExternalInput").ap()
    bq = nc.dram_tensor("bq", [128, NH], F32, kind="ExternalInput").ap()
    bk = nc.dram_tensor("bk", [128, 1], F32, kind="ExternalInput").ap()
    bv = nc.dram_tensor("bv", [128, 1], F32, kind="ExternalInput").ap()
    out4 = nc.dram_tensor("out4", [NLQ, NH, 128, 512], F32, kind="ExternalOutput").ap()
    with tile.TileContext(nc) as tc, ExitStack() as ctx:
        build_kernel(ctx, tc, xtb, wq8, wk8, wv8, bq, bk, bv, out4)
    nc.compile()
    _NC_CACHE = nc
    return nc


def _bf16(a):
    return np.ascontiguousarray(a.astype(ml_dtypes.bfloat16))


def _pack_pairs(wT, ncols):
    """wT [D, ncols] -> [D // 256, 128, 2 * ncols]: d-chunks 2kk, 2kk+1 side
    by side so every partition line is one contiguous >=2KB row."""
    return np.ascontiguousarray(
        wT.reshape(-1, 2, 128, ncols).transpose(0, 2, 1, 3).reshape(-1, 128, 2 * ncols)
    )


def _pack_oct(wT):
    """wT [D, 128] -> [2, 128, 1024]: 8 d-chunks of 128 cols side by side."""
    return np.ascontiguousarray(
        wT.reshape(2, 8, 128, 128).transpose(0, 2, 1, 3).reshape(2, 128, 1024)
    )


def make_in_maps(x, Wq_w, Wq_b, Wk_w, Wk_b, Wv_w, Wv_b):
    """Host-side sharding/relayout. Returns one input map per core."""
    x = np.asarray(x, dtype=np.float32)
    Wq_w = np.asarray(Wq_w, dtype=np.float32)
    Wq_b = np.asarray(Wq_b, dtype=np.float32)
    Wk_w = np.asarray(Wk_w, dtype=np.float32)
    Wk_b = np.asarray(Wk_b, dtype=np.float32)
    Wv_w = np.asarray(Wv_w, dtype=np.float32)
    Wv_b = np.asarray(Wv_b, dtype=np.float32)

    xtbs = [_bf16(x[b].T) for b in range(B)]
    wk8 = _pack_oct(_bf16(Wk_w.T))
    wv8 = _pack_oct(_bf16(Wv_w.T))
    bk = np.ascontiguousarray(Wk_b.reshape(128, 1))
    bv = np.ascontiguousarray(Wv_b.reshape(128, 1))
    in_maps = []
    for c in range(N_CORES):
        b, g = divmod(c, B * 2)  # b = c // 4, g = c % 4
        wq8_g = _pack_pairs(_bf16(Wq_w[g * QC:(g + 1) * QC, :].T), QC)
        bq_g = np.ascontiguousarray(Wq_b[g * QC:(g + 1) * QC].reshape(NH, 128).T)
        in_maps.append(
            {
                "xtb": xtbs[b],
                "wq8": wq8_g,
                "wk8": wk8,
                "wv8": wv8,
                "bq": bq_g,
                "bk": bk,
                "bv": bv,
            }
        )
    return in_maps


def assemble_output(results):
    out = np.empty((B, L, D), dtype=np.float32)
    for c in range(N_CORES):
        b, g = divmod(c, B * 2)
        o4 = np.asarray(results[c]["out4"])  # [lq, h, d, q]
        out[b, :, g * QC:(g + 1) * QC] = o4.transpose(0, 3, 1, 2).reshape(L, QC)
    return out


def kernel(**inputs) -> np.ndarray:
    nc = build_nc()
    in_maps = make_in_maps(**inputs)
    res = run_bass_kernel_spmd(nc, in_maps, core_ids=list(range(N_CORES)))
    return assemble_output(res.results)
